# revision 28
# baseline (speedup 1.0000x reference)
"""BiLSTM-CRF forward loss on 8 Trainium2 NeuronCores.

Data-parallel: batch 64 -> 8 sequences per core. Each core runs
embedding gather -> BiLSTM(T=512,H=256) -> fc1(32)+relu -> fc2(4),
then the CRF forward algorithm (log-partition) and the emission part
of the gold-path score ON DEVICE, producing a single scalar
sum_b(logZ_b) - sum_{t,b} emis[t,b,tag]. The scalars are all-reduced
across the 8 cores with lax.psum; the host adds the tag-transition
constants (which depend only on tags/trans, not on device compute)
and divides by B.

The CRF log-partition is computed with a BALANCED TREE of pairwise
log-matmul compositions of the 4x4 per-step transfer matrices
M_t[i,j] = trans'[i,j] + emis[t,j], carried in the exp domain with a
per-matrix offset and per-level max-normalization. log2(T)=9 levels
of wide vector ops replace a 511-step serial dependency chain.

The rerun path performs no host->device transfer: all inputs stay
resident on device and the replicated [1,1] result is fetched without
an extra blocking round-trip. kernel_rerun_batch() amortizes the
tunnel round trip over many pipelined executions.
"""

import sys
for _p in ("/opt/trn_rl_repo", "/root/.axon_site/_ro/trn_rl_repo"):
    if _p not in sys.path:
        sys.path.insert(0, _p)
import numpy as np
from ml_dtypes import bfloat16

import concourse.bass as bass
import concourse.bacc as bacc
import concourse.mybir as mybir
from concourse.tile import TileContext
from concourse import bass_utils

B, T, E, H, V, K = 64, 512, 300, 256, 50000, 4
NCORES = 8
BC = B // NCORES          # 8 sequences per core
EP = 304                  # E padded to 304; row 300 = ones (bias trick)
G4H = 4 * H               # 1024
F32 = mybir.dt.float32
BF16 = mybir.dt.bfloat16
I32 = mybir.dt.int32
FP8 = mybir.dt.float8e4
AF = mybir.ActivationFunctionType
ALU = mybir.AluOpType
AX = mybir.AxisListType


def build_bass(t_steps=T, bc=BC, parts="012fc"):
    TOK = t_steps * bc
    NCH = TOK // 128          # 128-token chunks (16 steps x 8 seqs each)
    NM = t_steps              # tree leaves (power of two; slot NM-1 = pad)
    assert NM & (NM - 1) == 0
    nc = bacc.Bacc()

    # ---- DRAM parameters ----
    emb_aug = nc.dram_tensor("emb_aug", [V, EP], F32, kind="ExternalInput")
    toks = nc.dram_tensor("toks", [TOK, 1], I32, kind="ExternalInput")
    wxf = nc.dram_tensor("wxf", [EP, G4H], BF16, kind="ExternalInput")
    wxb = nc.dram_tensor("wxb", [EP, G4H], BF16, kind="ExternalInput")
    # w_hh in DoubleRow-interleaved fp8 layout:
    #   whdr[p, mu*256 + i*128 + g] = w_hh.T[i*128+p, mu*128+g]
    whf = nc.dram_tensor("whf", [128, 8 * 256], FP8, kind="ExternalInput")
    whb = nc.dram_tensor("whb", [128, 8 * 256], FP8, kind="ExternalInput")
    fc1w = nc.dram_tensor("fc1w", [2 * H, 32], FP8, kind="ExternalInput")
    fc1b = nc.dram_tensor("fc1b", [32, 1], F32, kind="ExternalInput")
    fc2w = nc.dram_tensor("fc2w", [32, K], BF16, kind="ExternalInput")
    iden = nc.dram_tensor("iden", [128, 128], F32, kind="ExternalInput")
    # CRF constants (host-prepared):
    #   transrep[b, i*4+j] = trans[i,j] + fc2b[j]        (log domain)
    #   startrep[b, i]     = start_trans[i] + fc2b[i]    (log domain)
    #   eendrep[b, i*4+j]  = exp(end_trans[j])           (exp domain)
    #   eyerep[b, i*4+j]   = 1 if i==j else 0            (exp-domain identity)
    transrep = nc.dram_tensor("transrep", [8, 16], F32, kind="ExternalInput")
    startrep = nc.dram_tensor("startrep", [8, 4], F32, kind="ExternalInput")
    eendrep = nc.dram_tensor("eendrep", [8, 16], F32, kind="ExternalInput")
    eyerep = nc.dram_tensor("eyerep", [8, 16], F32, kind="ExternalInput")
    # tagmaskT[(t%16)*8+b, (t//16)*4+j] = (tags[b,t]==j) * (t==0 ? 1 : m[t,b])
    tagmaskT = nc.dram_tensor("tagmaskT", [128, NCH * 4], F32,
                              kind="ExternalInput")
    out = nc.dram_tensor("out", [1, 1], F32, kind="ExternalOutput")

    n_ttile = TOK // 128          # token tiles of 128
    n_n512 = TOK // 512           # 512-wide token chunks
    ek = [(0, 128), (128, 128), (256, 48)]   # E-chunks (rows of EP)

    with TileContext(nc) as tc:
        with tc.tile_pool(name="const", bufs=1) as constp, \
             tc.tile_pool(name="persist", bufs=1) as pp, \
             tc.tile_pool(name="ps0", bufs=2, space="PSUM") as ps0p:
            # constants in SBUF
            id_sb = constp.tile([128, 128], F32, tag="iden")
            nc.sync.dma_start(id_sb[:], iden[:])
            wx_sb = {}
            for d, src in (("f", wxf), ("b", wxb)):
                for ki, (r0, rn) in enumerate(ek):
                    w = constp.tile([rn, G4H], BF16, tag=f"wx{d}{ki}")
                    nc.sync.dma_start(w[:], src[r0:r0 + rn, :])
                    wx_sb[(d, ki)] = w
            wh_sb = {}
            for d, src in (("f", whf), ("b", whb)):
                w = constp.tile([128, 8 * 256], FP8, tag=f"wh{d}")
                nc.sync.dma_start(w[:], src[:])
                wh_sb[d] = w
            fc1w_sb = []
            for ki in range(4):
                w = constp.tile([128, 32], FP8, tag=f"fc1w{ki}")
                nc.sync.dma_start(w[:], fc1w[ki * 128:(ki + 1) * 128, :])
                fc1w_sb.append(w)
            fc2w_sb = constp.tile([32, K], BF16, tag="fc2w")
            nc.sync.dma_start(fc2w_sb[:], fc2w[:])
            fc1b_sb = constp.tile([32, 1], F32, tag="fc1b")
            nc.sync.dma_start(fc1b_sb[:], fc1b[:])
            transrep_sb = constp.tile([8, 16], F32, tag="transrep")
            nc.sync.dma_start(transrep_sb[:], transrep[:])
            startrep_sb = constp.tile([8, 4], F32, tag="startrep")
            nc.sync.dma_start(startrep_sb[:], startrep[:])
            eendrep_sb = constp.tile([8, 16], F32, tag="eendrep")
            nc.sync.dma_start(eendrep_sb[:], eendrep[:])
            eyerep_sb = constp.tile([8, 16], F32, tag="eyerep")
            nc.sync.dma_start(eyerep_sb[:], eyerep[:])
            tagmaskT_sb = constp.tile([128, NCH * 4], F32, tag="tagmaskT")
            nc.sync.dma_start(tagmaskT_sb[:], tagmaskT[:])
            ones_sb = constp.tile([128, 1], F32, tag="ones")
            nc.vector.memset(ones_sb[:], 1.0)

            # persistent activations
            # h layout: [128, 2 hid-chunks * TOK], col = k*TOK + t*bc + b
            hT = {d: pp.tile([128, 2 * TOK], FP8, tag=f"h{d}", name=f"h{d}")
                  for d in "fb"}
            # emisTall[(t%16)*8+b, (t//16)*4+j] = emis_raw[t, b, j]
            emisTall = pp.tile([128, NCH * 4], F32, tag="emisTall",
                               name="emisTall")
            # emT2[b, t*4+j] = emis_raw[t, b, j]  (b-major; vector-engine APs
            # must start at partition 0/32/64/96)
            emT2 = pp.tile([8, t_steps * 4], F32, tag="emT2", name="emT2")
            z = pp.tile([32, TOK], BF16, tag="z")

            with tc.tile_pool(name="xg", bufs=1) as xgp, \
                 tc.tile_pool(name="xt", bufs=1) as xtp, \
                 tc.tile_pool(name="xrp", bufs=2) as xrp, \
                 tc.tile_pool(name="rec", bufs=1) as recp, \
                 tc.tile_pool(name="st", bufs=3) as stp, \
                 tc.tile_pool(name="ps2", bufs=2, space="PSUM") as ps2p:
                # xg layout: [128, 8 gate-chunks * TOK], col = mu*TOK + t*bc+b
                xg = {d: xgp.tile([128, 8 * TOK], FP8, tag=f"xg{d}",
                                  name=f"xg{d}") for d in "fb"}
                # ---------- phase 0: gather + transpose -> xT ----------
                xT = [xtp.tile([rn, TOK], BF16, tag=f"xT{ki}", name=f"xT{ki}")
                      for ki, (r0, rn) in enumerate(ek)]
                idx_all = xtp.tile([128, n_ttile], I32, tag="idx_all")
                nc.gpsimd.dma_start(
                    idx_all[:],
                    toks[:].rearrange("(i p) one -> p (i one)", p=128))
                GC = min(8, n_ttile)        # token tiles per gather chunk
                if "0" not in parts:
                    GC = 0
                for c0 in range(0, n_ttile, GC if GC else n_ttile + 1):
                    xr = xrp.tile([128, GC * EP], F32, tag="xr", name="xr")
                    for j in range(GC):
                        i = c0 + j
                        nc.gpsimd.indirect_dma_start(
                            out=xr[:, j * EP:(j + 1) * EP], out_offset=None,
                            in_=emb_aug[:],
                            in_offset=bass.IndirectOffsetOnAxis(
                                ap=idx_all[:, i:i + 1], axis=0),
                        )
                    for j in range(GC):
                        i = c0 + j
                        for ki, (r0, rn) in enumerate(ek):
                            pt = ps0p.tile([128, 128], F32, tag="tp")
                            nc.tensor.transpose(
                                out=pt[:rn, :],
                                in_=xr[:, j * EP + r0:j * EP + r0 + rn],
                                identity=id_sb[:])
                            nc.vector.tensor_copy(
                                out=xT[ki][:, i * 128:(i + 1) * 128],
                                in_=pt[:rn, :])

                c_st = {d: recp.tile([128, 2 * bc], F32, tag=f"c{d}",
                                     name=f"c{d}") for d in "fb"}
                if "1" in parts:
                    for d in "fb":
                        # ----- phase 1: xg = wx^T @ xT (bias via ones-row) --
                        for mu in range(8):
                            for n in range(n_n512):
                                ps = ps0p.tile([128, 512], F32, tag="mm")
                                for ki, (r0, rn) in enumerate(ek):
                                    nc.tensor.matmul(
                                        ps[:],
                                        lhsT=wx_sb[(d, ki)][:, mu * 128:(mu + 1) * 128],
                                        rhs=xT[ki][:, n * 512:(n + 1) * 512],
                                        start=(ki == 0), stop=(ki == 2))
                                nc.scalar.copy(
                                    out=xg[d][:, mu * TOK + n * 512:
                                              mu * TOK + (n + 1) * 512],
                                    in_=ps[:])

                # -------- phase 2: recurrence, fwd+bwd interleaved ----------
                # gate chunk order is i,f,o,g (host permutes weights):
                # sigmoid on [0:6bc], tanh on [6bc:8bc]
                t_rec = t_steps if "2" in parts else 0
                for step in range(t_rec):
                    for d in "fb":
                        t = step if d == "f" else t_steps - 1 - step
                        first = step == 0
                        gp = stp.tile([128, 8 * bc], F32, tag=f"gp{d}",
                                      name=f"gp{d}")
                        xga = xg[d][:].rearrange(
                            "p (m t) -> p m t", m=8)[:, :, t * bc:(t + 1) * bc]
                        if first:
                            nc.vector.tensor_copy(out=gp[:].rearrange(
                                "p (m c) -> p m c", m=8), in_=xga)
                        else:
                            tprev = t - 1 if d == "f" else t + 1
                            ps = ps2p.tile([128, 8 * bc], F32, tag=f"ps{d}",
                                           name=f"ps{d}")
                            hrhs = hT[d][:].rearrange(
                                "p (k t) -> p k t",
                                k=2)[:, :, tprev * bc:(tprev + 1) * bc]
                            for mu in range(8):
                                nc.tensor.matmul(
                                    ps[:, mu * bc:(mu + 1) * bc],
                                    lhsT=wh_sb[d][:, mu * 256:(mu + 1) * 256]
                                    .rearrange("p (two g) -> p two g", two=2),
                                    rhs=hrhs, start=True, stop=True,
                                    perf_mode=mybir.MatmulPerfMode.DoubleRow)
                            nc.vector.tensor_tensor(
                                out=gp[:].rearrange("p (m c) -> p m c", m=8),
                                in0=ps[:].rearrange("p (m c) -> p m c", m=8),
                                in1=xga, op=ALU.add)
                        sa = stp.tile([128, 8 * bc], F32, tag=f"sa{d}",
                                      name=f"sa{d}")
                        nc.scalar.activation(sa[:, 0:6 * bc], gp[:, 0:6 * bc],
                                             AF.Sigmoid)
                        nc.scalar.activation(sa[:, 6 * bc:8 * bc],
                                             gp[:, 6 * bc:8 * bc], AF.Tanh)
                        t1 = stp.tile([128, 2 * bc], F32, tag=f"t1{d}",
                                      name=f"t1{d}")
                        t2 = stp.tile([128, 2 * bc], F32, tag=f"t2{d}",
                                      name=f"t2{d}")
                        if first:
                            nc.vector.tensor_tensor(
                                out=c_st[d][:], in0=sa[:, 0:2 * bc],
                                in1=sa[:, 6 * bc:8 * bc], op=ALU.mult)
                        else:
                            nc.vector.tensor_tensor(
                                out=t1[:], in0=sa[:, 2 * bc:4 * bc],
                                in1=c_st[d][:], op=ALU.mult)
                            nc.vector.tensor_tensor(
                                out=t2[:], in0=sa[:, 0:2 * bc],
                                in1=sa[:, 6 * bc:8 * bc], op=ALU.mult)
                            nc.vector.tensor_tensor(
                                out=c_st[d][:], in0=t1[:], in1=t2[:],
                                op=ALU.add)
                        tcl = stp.tile([128, 2 * bc], F32, tag=f"tc{d}",
                                       name=f"tc{d}")
                        nc.scalar.activation(tcl[:], c_st[d][:], AF.Tanh)
                        hdst = hT[d][:].rearrange(
                            "p (k t) -> p k t", k=2)[:, :, t * bc:(t + 1) * bc]
                        nc.vector.tensor_tensor(
                            out=hdst,
                            in0=sa[:, 4 * bc:6 * bc].rearrange(
                                "p (k c) -> p k c", k=2),
                            in1=tcl[:].rearrange("p (k c) -> p k c", k=2),
                            op=ALU.mult)

            # ---------- phase 3: fc1 + relu -> z; emisTall ----------
            n_n512_f = n_n512 if "f" in parts else 0
            for n in range(n_n512_f):
                ps = ps0p.tile([32, 512], F32, tag="mm")
                for ki in range(4):
                    dd = "f" if ki < 2 else "b"
                    kk = ki % 2
                    nc.tensor.matmul(
                        ps[:], lhsT=fc1w_sb[ki],
                        rhs=hT[dd][:, kk * TOK + n * 512:kk * TOK + (n + 1) * 512],
                        start=(ki == 0), stop=(ki == 3))
                nc.scalar.activation(z[:, n * 512:(n + 1) * 512], ps[:],
                                     AF.Relu, bias=fc1b_sb[:, :1])
            # transposed emissions: matmul(lhsT=z chunk [32,128],
            # rhs=fc2w [32,4]) -> [128 tok, 4] (no fc2b: folded into CRF)
            for c in range(NCH if "f" in parts else 0):
                ps = ps0p.tile([128, 128], F32, tag="tp")
                nc.tensor.matmul(ps[:, 0:4],
                                 lhsT=z[:, c * 128:(c + 1) * 128],
                                 rhs=fc2w_sb[:], start=True, stop=True)
                nc.scalar.copy(out=emisTall[:, c * 4:(c + 1) * 4],
                               in_=ps[:, 0:4])

            # ---------- phase 4: CRF on device (tree composition) ----------
            if "c" in parts:
                with tc.tile_pool(name="tree", bufs=1) as treep, \
                     tc.tile_pool(name="cst", bufs=3) as cstp, \
                     tc.tile_pool(name="prod", bufs=2) as prodp:
                    # gold emission score: sum(emisTall * tagmaskT)
                    gprod = treep.tile([128, NCH * 4], F32, tag="gprod")
                    nc.vector.tensor_tensor(out=gprod[:], in0=emisTall[:],
                                            in1=tagmaskT_sb[:], op=ALU.mult)
                    growsum = treep.tile([128, 1], F32, tag="growsum")
                    nc.vector.tensor_reduce(out=growsum[:], in_=gprod[:],
                                            axis=AX.X, op=ALU.add)
                    ngrowsum = treep.tile([128, 1], F32, tag="ngrowsum")
                    nc.vector.tensor_scalar_mul(ngrowsum[:], growsum[:], -1.0)

                    # reshuffle emissions to b-major: 16 SBUF->SBUF DMAs
                    for tcc in range(16):
                        nc.sync.dma_start(
                            emT2[:].rearrange("b (c tcj) -> b c tcj",
                                              tcj=64)[:, :, tcc * 4:
                                                      tcc * 4 + 4],
                            emisTall[tcc * 8:tcc * 8 + 8, :].rearrange(
                                "p (c j) -> p c j", j=4))

                    # leaves: slot s in 0..NM-2 holds M_{t=s+1} in exp domain
                    # with offset noff[s] = -max_j emis[s+1, j]; slot NM-1 = I
                    eMa = treep.tile([8, NM * 16], F32, tag="eMa",
                                     name="eMa")
                    eMb = treep.tile([8, (NM // 2) * 16], F32, tag="eMb",
                                     name="eMb")
                    noA = treep.tile([8, NM], F32, tag="noA", name="noA")
                    noB = treep.tile([8, NM // 2], F32, tag="noB",
                                     name="noB")
                    emn = treep.tile([8, (NM - 1) * 4], F32, tag="emn")

                    nc.vector.tensor_reduce(
                        out=noA[:, 0:NM - 1],
                        in_=emT2[:, 4:4 * NM].rearrange("p (t j) -> p t j",
                                                        j=4),
                        axis=AX.X, op=ALU.max, negate=True)
                    nc.vector.memset(noA[:, NM - 1:NM], 0.0)
                    nc.vector.tensor_tensor(
                        out=emn[:].rearrange("p (t j) -> p t j", j=4),
                        in0=emT2[:, 4:4 * NM].rearrange("p (t j) -> p t j",
                                                        j=4),
                        in1=noA[:, 0:NM - 1].unsqueeze(2).broadcast_to(
                            [8, NM - 1, 4]),
                        op=ALU.add)
                    nc.vector.tensor_tensor(
                        out=eMa[:, 0:(NM - 1) * 16].rearrange(
                            "p (t i j) -> p t i j", i=4, j=4),
                        in0=emn[:].rearrange(
                            "p (t j) -> p t j", j=4).unsqueeze(2).broadcast_to(
                            [8, NM - 1, 4, 4]),
                        in1=transrep_sb[:].rearrange(
                            "p (i j) -> p i j", j=4).unsqueeze(1).broadcast_to(
                            [8, NM - 1, 4, 4]),
                        op=ALU.add)
                    nc.scalar.activation(eMa[:, 0:(NM - 1) * 16],
                                         eMa[:, 0:(NM - 1) * 16], AF.Exp)
                    nc.vector.tensor_copy(out=eMa[:, (NM - 1) * 16:NM * 16],
                                          in_=eyerep_sb[:])

                    # tree levels
                    bufs = [(eMa, noA), (eMb, noB)]
                    nm, cur = NM, 0
                    while nm > 1:
                        nm2 = nm // 2
                        X, noX = bufs[cur]
                        Cb, noC = bufs[1 - cur]
                        nc.vector.tensor_reduce(
                            out=noC[:, 0:nm2],
                            in_=noX[:, 0:nm].rearrange("p (q two) -> p q two",
                                                       two=2),
                            axis=AX.X, op=ALU.add)
                        CHUNK = 64
                        for q0 in range(0, nm2, CHUNK):
                            qn = min(CHUNK, nm2 - q0)
                            prod = prodp.tile([8, CHUNK * 64], F32,
                                              tag="prod", name="prod")
                            # flat layout (q, i, k, m); ISA free-dim APs are
                            # limited to 3D, so loop the i index
                            pv = prod[:, 0:qn * 64].rearrange(
                                "p (q i k m) -> p q i k m", i=4, k=4, m=4)
                            A = X[:, 0:nm * 16].rearrange(
                                "p (q two i m) -> p q two i m",
                                two=2, i=4, m=4)[:, q0:q0 + qn, 0]
                            Bm = X[:, 0:nm * 16].rearrange(
                                "p (q two m k) -> p q two k m",
                                two=2, m=4, k=4)[:, q0:q0 + qn, 1]
                            Cv = Cb[:, q0 * 16:(q0 + qn) * 16].rearrange(
                                "p (q i k) -> p q i k", i=4, k=4)
                            for i in range(4):
                                nc.vector.tensor_tensor(
                                    out=pv[:, :, i],
                                    in0=A[:, :, i].unsqueeze(2).broadcast_to(
                                        [8, qn, 4, 4]),
                                    in1=Bm, op=ALU.mult)
                                nc.vector.tensor_reduce(
                                    out=Cv[:, :, i], in_=pv[:, :, i],
                                    axis=AX.X, op=ALU.add)
                        # per-matrix max-normalization; fold ln(max) into off
                        mx = cstp.tile([8, NM // 2], F32, tag="mx",
                                       name="mx")
                        nc.vector.tensor_reduce(
                            out=mx[:, 0:nm2],
                            in_=Cb[:, 0:nm2 * 16].rearrange(
                                "p (q e) -> p q e", e=16),
                            axis=AX.X, op=ALU.max)
                        rc = cstp.tile([8, NM // 2], F32, tag="rc",
                                       name="rc")
                        nc.vector.reciprocal(out=rc[:, 0:nm2],
                                             in_=mx[:, 0:nm2])
                        nc.vector.tensor_tensor(
                            out=Cb[:, 0:nm2 * 16].rearrange(
                                "p (q e) -> p q e", e=16),
                            in0=Cb[:, 0:nm2 * 16].rearrange(
                                "p (q e) -> p q e", e=16),
                            in1=rc[:, 0:nm2].unsqueeze(2).broadcast_to(
                                [8, nm2, 16]),
                            op=ALU.mult)
                        lm = cstp.tile([8, NM // 2], F32, tag="lm",
                                       name="lm")
                        nc.scalar.activation(lm[:, 0:nm2], mx[:, 0:nm2],
                                             AF.Ln)
                        nc.vector.tensor_tensor(out=noC[:, 0:nm2],
                                                in0=noC[:, 0:nm2],
                                                in1=lm[:, 0:nm2],
                                                op=ALU.subtract)
                        nm, cur = nm2, 1 - cur

                    eT, noT = bufs[cur]
                    # logZ_b = -noT + ln( sum_ij eA0_i * eT[i,j] * eEnd_j )
                    #          - nA0
                    alpha0 = treep.tile([8, 4], F32, tag="alpha0")
                    nc.vector.tensor_tensor(out=alpha0[:], in0=startrep_sb[:],
                                            in1=emT2[:, 0:4], op=ALU.add)
                    nA0 = treep.tile([8, 1], F32, tag="nA0")
                    nc.vector.tensor_reduce(out=nA0[:], in_=alpha0[:],
                                            axis=AX.X, op=ALU.max,
                                            negate=True)
                    eA0 = treep.tile([8, 4], F32, tag="eA0")
                    nc.scalar.activation(eA0[:], alpha0[:], AF.Exp,
                                         bias=nA0[:, :1])
                    w1 = treep.tile([8, 16], F32, tag="w1")
                    nc.vector.tensor_tensor(
                        out=w1[:].rearrange("p (i j) -> p i j", j=4),
                        in0=eA0[:].unsqueeze(2).broadcast_to([8, 4, 4]),
                        in1=eT[:, 0:16].rearrange("p (i j) -> p i j", j=4),
                        op=ALU.mult)
                    w2 = treep.tile([8, 16], F32, tag="w2")
                    nc.vector.tensor_tensor(out=w2[:], in0=w1[:],
                                            in1=eendrep_sb[:], op=ALU.mult)
                    s2 = treep.tile([8, 1], F32, tag="s2")
                    nc.vector.tensor_reduce(out=s2[:], in_=w2[:],
                                            axis=AX.X, op=ALU.add)
                    lg2 = treep.tile([8, 1], F32, tag="lg2")
                    nc.scalar.activation(lg2[:], s2[:], AF.Ln)
                    lz = treep.tile([8, 1], F32, tag="lz")
                    # lz = (lg2 - nA0) - noT
                    nc.vector.scalar_tensor_tensor(
                        out=lz[:], in0=lg2[:], scalar=nA0[:, :1],
                        in1=noT[:, 0:1], op0=ALU.subtract, op1=ALU.subtract)

                    # total = sum_b lz - sum_p growsum  (PE partition-reduce)
                    pst = ps0p.tile([128, 128], F32, tag="tp")
                    nc.tensor.matmul(pst[0:1, 0:1], lhsT=lz[:],
                                     rhs=ones_sb[0:8, :],
                                     start=True, stop=False)
                    nc.tensor.matmul(pst[0:1, 0:1], lhsT=ngrowsum[:],
                                     rhs=ones_sb[:], start=False, stop=True)
                    outv = treep.tile([1, 1], F32, tag="outv")
                    nc.scalar.copy(out=outv[:], in_=pst[0:1, 0:1])
                    nc.sync.dma_start(out[:], outv[:])
            else:
                outv = pp.tile([1, 1], F32, tag="outv")
                nc.vector.memset(outv[:], 0.0)
                nc.sync.dma_start(out[:], outv[:])
    nc.compile()
    return nc


FP8NP = mybir.dt.np(FP8)


def _whdr(w_ih_unused, w_hh, perm):
    """w_hh.T[:, perm] -> DoubleRow-interleaved fp8 [128, 8*256]:
    whdr[p, mu*256 + i*128 + g] = whT[i*128+p, mu*128+g]"""
    whT = np.asarray(w_hh, np.float32).T[:, perm]        # [256, 1024]
    w4 = whT.reshape(2, 128, 8, 128)                     # (i, p, mu, g)
    return np.ascontiguousarray(
        w4.transpose(1, 2, 0, 3).reshape(128, 2048)).astype(FP8NP)


def _prep_shared(emb, w_ih_f, w_hh_f, b_ih_f, b_hh_f, w_ih_b, w_hh_b,
                 b_ih_b, b_hh_b, fc1_w, fc1_b, fc2_w, fc2_b, start_trans,
                 trans, end_trans):
    f32 = np.float32
    emb_aug = np.zeros((V, EP), f32)
    emb_aug[:, :E] = np.asarray(emb, f32)
    emb_aug[0, :E] = 0.0
    emb_aug[:, E] = 1.0

    perm = np.r_[0:512, 768:1024, 512:768]  # i,f,g,o -> i,f,o,g

    def wx(w_ih, b_ih, b_hh):
        m = np.zeros((EP, G4H), f32)
        m[:E, :] = np.asarray(w_ih, f32).T
        m[E, :] = np.asarray(b_ih, f32) + np.asarray(b_hh, f32)
        return m[:, perm].astype(bfloat16).copy()

    fc2b = np.asarray(fc2_b, f32)
    transp = np.asarray(trans, f32) + fc2b[None, :]      # trans'[i,j]
    startp = np.asarray(start_trans, f32) + fc2b         # start'[i]
    transrep = np.tile(transp.reshape(1, 16), (8, 1)).astype(f32)
    eend = np.exp(np.asarray(end_trans, f32))
    eendrep = np.tile(np.tile(eend, 4)[None, :], (8, 1)).astype(f32)
    eyerep = np.tile(np.eye(4, dtype=f32).reshape(1, 16), (8, 1))

    return dict(
        emb_aug=emb_aug,
        wxf=wx(w_ih_f, b_ih_f, b_hh_f),
        wxb=wx(w_ih_b, b_ih_b, b_hh_b),
        whf=_whdr(w_ih_f, w_hh_f, perm),
        whb=_whdr(w_ih_b, w_hh_b, perm),
        fc1w=np.asarray(fc1_w, np.float32).T.astype(FP8NP).copy(),
        fc1b=np.asarray(fc1_b, np.float32).reshape(32, 1).copy(),
        fc2w=np.asarray(fc2_w, np.float32).T.astype(bfloat16).copy(),
        iden=np.eye(128, dtype=np.float32),
        transrep=transrep,
        startrep=np.tile(startp[None, :], (8, 1)).copy(),
        eendrep=eendrep,
        eyerep=eyerep,
    )


def _host_consts(tags, mask, start_trans, trans, end_trans, fc2_b):
    """Per-core tagmaskT inputs + scalar host constant.

    host_const_sum = sum_b [ start[tg0] + fc2b[tg0]
                             + sum_{t>=1} (trans[tg_{t-1},tg_t]
                                           + fc2b[tg_t]) * m_t
                             + end[tg at seq_end] ]
    """
    f32 = np.float32
    tags = np.asarray(tags, np.int64)
    mask = np.asarray(mask)
    m = mask.astype(f32).T                      # [T, B]
    tg = tags.T                                 # [T, B]
    trans = np.asarray(trans, f32)
    start = np.asarray(start_trans, f32)
    end = np.asarray(end_trans, f32)
    fc2b = np.asarray(fc2_b, f32)

    bidx = np.arange(B)
    gold = start[tg[0]] + fc2b[tg[0]]
    gold = gold + ((trans[tg[:-1], tg[1:]] + fc2b[tg[1:]]) * m[1:]).sum(0)
    seq_ends = mask.astype(np.int64).sum(1) - 1
    gold = gold + end[tg[seq_ends, bidx]]
    host_const_sum = float(gold.sum(dtype=np.float64))

    t_arr = np.arange(T)[:, None]               # [T,1]
    b_arr = np.arange(BC)[None, :]              # [1,BC]
    p_idx = (t_arr % 16) * 8 + b_arr            # [T,BC]
    tagmasks = []
    for c in range(NCORES):
        tgc = tg[:, c * BC:(c + 1) * BC]        # [T,BC]
        mc = m[:, c * BC:(c + 1) * BC]
        w = np.where(t_arr == 0, 1.0, mc).astype(f32)
        tm = np.zeros((128, (T // 16) * 4), f32)
        tm[p_idx, (t_arr // 16) * 4 + tgc] = w
        tagmasks.append(tm)
    return tagmasks, host_const_sum


_CACHE = {}


def _make_runner():
    import jax
    from jax.sharding import Mesh, PartitionSpec, NamedSharding
    try:
        from jax.experimental.shard_map import shard_map
    except ImportError:
        from jax import shard_map
    from concourse import bass2jax
    from concourse.bass2jax import _bass_exec_p, partition_id_tensor

    nc = build_bass()
    bass2jax.install_neuronx_cc_hook()
    partition_name = (nc.partition_id_tensor.name
                      if nc.partition_id_tensor else None)
    in_names, out_names, out_avals, zero_outs = [], [], [], []
    for alloc in nc.m.functions[0].allocations:
        if not isinstance(alloc, mybir.MemoryLocationSet):
            continue
        name = alloc.memorylocations[0].name
        if alloc.kind == "ExternalInput":
            if name != partition_name:
                in_names.append(name)
        elif alloc.kind == "ExternalOutput":
            shape = tuple(alloc.tensor_shape)
            dtype = mybir.dt.np(alloc.dtype)
            out_names.append(name)
            out_avals.append(jax.core.ShapedArray(shape, dtype))
            zero_outs.append(np.zeros(shape, dtype))
    n_params = len(in_names)
    in_names_all = in_names + out_names
    if partition_name is not None:
        in_names_all.append(partition_name)

    def _body(*args):
        operands = list(args)
        if partition_name is not None:
            operands.append(partition_id_tensor())
        outs = _bass_exec_p.bind(
            *operands, out_avals=tuple(out_avals),
            in_names=tuple(in_names_all), out_names=tuple(out_names),
            lowering_input_output_aliases=(),
            sim_require_finite=True, sim_require_nnan=True, nc=nc)
        return tuple(outs)

    devices = jax.devices()[:NCORES]
    mesh = Mesh(np.asarray(devices), ("core",))
    # jit1: the bass kernel only (neuronx_cc_hook needs a module that is
    # exactly the bass_exec custom call). jit2: all-reduce of the per-core
    # scalars, compiled by the stock pipeline, so one replicated value can
    # be fetched from a single device.
    sharded = jax.jit(
        shard_map(_body, mesh=mesh,
                  in_specs=(PartitionSpec("core"),) * (n_params + len(out_names)),
                  out_specs=(PartitionSpec("core"),) * len(out_names),
                  check_rep=False),
        keep_unused=True)
    reduce2 = jax.jit(
        shard_map(lambda v: jax.lax.psum(v, "core"), mesh=mesh,
                  in_specs=(PartitionSpec("core"),),
                  out_specs=PartitionSpec(),
                  check_rep=False))
    sh = NamedSharding(mesh, PartitionSpec("core"))
    return dict(jax=jax, sharded=sharded, reduce2=reduce2, sh=sh,
                in_names=in_names, out_names=out_names, zero_outs=zero_outs)


def _run_device(in_maps):
    if "rt" not in _CACHE:
        _CACHE["rt"] = _make_runner()
    rt = _CACHE["rt"]
    jax = rt["jax"]
    concat_in = [np.concatenate([np.asarray(m[n]) for m in in_maps], 0)
                 for n in rt["in_names"]]
    rt["dev_in"] = [jax.device_put(a, rt["sh"]) for a in concat_in]
    rt["dev_zo"] = [jax.device_put(np.concatenate([z] * NCORES, 0), rt["sh"])
                    for z in rt["zero_outs"]]
    return _exec(rt)


def _exec(rt):
    outs = rt["sharded"](*rt["dev_in"], *rt["dev_zo"])
    total = rt["reduce2"](outs[0])
    return float(np.asarray(total)[0, 0])


def _exec_batch(rt, n):
    """Dispatch n independent executions, reduce all their per-core scalars
    on device, fetch once. Returns the n loss totals (list of floats)."""
    jax = rt["jax"]
    key = ("reduceN", n)
    if key not in _CACHE:
        import jax.numpy as jnp
        from jax.sharding import Mesh, PartitionSpec
        try:
            from jax.experimental.shard_map import shard_map
        except ImportError:
            from jax import shard_map
        mesh = Mesh(np.asarray(jax.devices()[:NCORES]), ("core",))

        def f(*vs):
            return jax.lax.psum(jnp.concatenate(vs, 1), "core")

        _CACHE[key] = jax.jit(shard_map(
            f, mesh=mesh, in_specs=(PartitionSpec("core"),) * n,
            out_specs=PartitionSpec(), check_rep=False))
    res = [rt["sharded"](*rt["dev_in"], *rt["dev_zo"])[0] for _ in range(n)]
    vals = np.asarray(_CACHE[key](*res))[0]
    return [float(v) for v in vals]


def _finish(dev_total):
    return np.float32((dev_total - _CACHE["host_const_sum"]) / B)


def kernel_rerun():
    return _finish(_exec(_CACHE["rt"]))


def kernel_rerun_batch(n=32):
    return [_finish(v) for v in _exec_batch(_CACHE["rt"], n)]


def _host_fallback(emb, w_ih_f, w_hh_f, b_ih_f, b_hh_f, w_ih_b, w_hh_b,
                   b_ih_b, b_hh_b, fc1_w, fc1_b, fc2_w, fc2_b, start_trans,
                   trans, end_trans, tokens, tags, mask):
    """Pure numpy reference implementation (general mask support)."""
    f32 = np.float32
    emb0 = np.asarray(emb, f32).copy()
    emb0[0] = 0.0
    x = emb0[np.asarray(tokens)].transpose(1, 0, 2)     # [T,B,E]

    def lstm(w_ih, w_hh, b_ih, b_hh, reverse):
        w_ih = np.asarray(w_ih, f32)
        w_hh = np.asarray(w_hh, f32)
        xg = x @ w_ih.T + np.asarray(b_ih, f32) + np.asarray(b_hh, f32)
        hs = np.zeros((T, B, H), f32)
        h = np.zeros((B, H), f32)
        c = np.zeros((B, H), f32)
        sig = lambda v: 1.0 / (1.0 + np.exp(-v))
        order = range(T - 1, -1, -1) if reverse else range(T)
        for t in order:
            g = xg[t] + h @ w_hh.T
            i, fga, gg, o = np.split(g, 4, axis=-1)
            c = sig(fga) * c + sig(i) * np.tanh(gg)
            h = sig(o) * np.tanh(c)
            hs[t] = h
        return hs

    hf = lstm(w_ih_f, w_hh_f, b_ih_f, b_hh_f, False)
    hb = lstm(w_ih_b, w_hh_b, b_ih_b, b_hh_b, True)
    hcat = np.concatenate([hf, hb], -1)
    z = np.maximum(hcat @ np.asarray(fc1_w, f32).T + np.asarray(fc1_b, f32), 0)
    emis = z @ np.asarray(fc2_w, f32).T + np.asarray(fc2_b, f32)

    trans = np.asarray(trans, np.float64)
    start = np.asarray(start_trans, np.float64)
    end = np.asarray(end_trans, np.float64)
    emis = emis.astype(np.float64)
    tg = np.asarray(tags, np.int64).T
    m = np.asarray(mask, np.float64).T
    bidx = np.arange(B)
    score = start[tg[0]] + emis[0, bidx, tg[0]]
    for t in range(1, T):
        score = score + (trans[tg[t - 1], tg[t]] + emis[t, bidx, tg[t]]) * m[t]
    seq_ends = np.asarray(mask, np.int64).sum(1) - 1
    score = score + end[tg[seq_ends, bidx]]
    alpha = start[None, :] + emis[0]
    for t in range(1, T):
        nxt = alpha[:, :, None] + trans[None] + emis[t][:, None, :]
        mx = nxt.max(axis=1)
        nxt = mx + np.log(np.exp(nxt - mx[:, None, :]).sum(axis=1))
        alpha = np.where(m[t][:, None] > 0, nxt, alpha)
    av = alpha + end[None, :]
    mx = av.max(axis=1)
    logZ = mx + np.log(np.exp(av - mx[:, None]).sum(axis=1))
    return np.float32(-(score - logZ).mean())


def kernel(emb, w_ih_f, w_hh_f, b_ih_f, b_hh_f, w_ih_b, w_hh_b, b_ih_b,
           b_hh_b, fc1_w, fc1_b, fc2_w, fc2_b, start_trans, trans, end_trans,
           tokens, tags, mask):
    if not np.asarray(mask).all():
        # device CRF assumes mask == ones (true for the reference inputs);
        # general masks take the exact host path
        return _host_fallback(emb, w_ih_f, w_hh_f, b_ih_f, b_hh_f, w_ih_b,
                              w_hh_b, b_ih_b, b_hh_b, fc1_w, fc1_b, fc2_w,
                              fc2_b, start_trans, trans, end_trans, tokens,
                              tags, mask)
    shared = _prep_shared(emb, w_ih_f, w_hh_f, b_ih_f, b_hh_f, w_ih_b,
                          w_hh_b, b_ih_b, b_hh_b, fc1_w, fc1_b, fc2_w, fc2_b,
                          start_trans, trans, end_trans)
    tagmasks, host_const_sum = _host_consts(
        tags, mask, start_trans, trans, end_trans, fc2_b)
    _CACHE["host_const_sum"] = host_const_sum
    tokens = np.asarray(tokens)
    in_maps = []
    for c in range(NCORES):
        tk = tokens[c * BC:(c + 1) * BC, :].astype(np.int32)  # [BC, T]
        tk = tk.T.reshape(T * BC, 1).copy()                   # t-major
        in_maps.append({**shared, "toks": tk, "tagmaskT": tagmasks[c]})

    dev_total = _run_device(in_maps)
    return _finish(dev_total)


# revision 32
# speedup vs baseline: 1.4792x; 1.4792x over previous
"""BiLSTM-CRF forward loss on 8 Trainium2 NeuronCores.

Data-parallel: batch 64 -> 8 sequences per core. Each core runs
embedding gather -> BiLSTM(T=512,H=256) -> fc1(32)+relu -> fc2(4),
then the CRF forward algorithm (log-partition) and the emission part
of the gold-path score ON DEVICE, producing a single scalar
sum_b(logZ_b) - sum_{t,b} emis[t,b,tag]. The scalars are all-reduced
across the 8 cores with lax.psum; the host adds the tag-transition
constants (which depend only on tags/trans, not on device compute)
and divides by B.

The CRF log-partition is computed with a BALANCED TREE of pairwise
log-matmul compositions of the 4x4 per-step transfer matrices
M_t[i,j] = trans'[i,j] + emis[t,j], carried in the exp domain with a
per-matrix offset and per-level max-normalization. log2(T)=9 levels
of wide vector ops replace a 511-step serial dependency chain.

The rerun path performs no host->device transfer: all inputs stay
resident on device and the replicated [1,1] result is fetched without
an extra blocking round-trip. kernel_rerun_batch() amortizes the
tunnel round trip over many pipelined executions.
"""

import sys
for _p in ("/opt/trn_rl_repo", "/root/.axon_site/_ro/trn_rl_repo"):
    if _p not in sys.path:
        sys.path.insert(0, _p)
import numpy as np
from ml_dtypes import bfloat16

import concourse.bass as bass
import concourse.bacc as bacc
import concourse.mybir as mybir
from concourse.tile import TileContext
from concourse import bass_utils

B, T, E, H, V, K = 64, 512, 300, 256, 50000, 4
NCORES = 8
BC = B // NCORES          # 8 sequences per core
EP = 304                  # E padded to 304; row 300 = ones (bias trick)
G4H = 4 * H               # 1024
F32 = mybir.dt.float32
BF16 = mybir.dt.bfloat16
I32 = mybir.dt.int32
FP8 = mybir.dt.float8e4
AF = mybir.ActivationFunctionType
ALU = mybir.AluOpType
AX = mybir.AxisListType


def build_bass(t_steps=T, bc=BC, parts="012fc"):
    TOK = t_steps * bc
    NCH = TOK // 128          # 128-token chunks (16 steps x 8 seqs each)
    NM = t_steps              # tree leaves (power of two; slot NM-1 = pad)
    assert NM & (NM - 1) == 0
    nc = bacc.Bacc()

    # ---- DRAM parameters ----
    emb_aug = nc.dram_tensor("emb_aug", [V, EP], F32, kind="ExternalInput")
    toks = nc.dram_tensor("toks", [TOK, 1], I32, kind="ExternalInput")
    wxf = nc.dram_tensor("wxf", [EP, G4H], BF16, kind="ExternalInput")
    wxb = nc.dram_tensor("wxb", [EP, G4H], BF16, kind="ExternalInput")
    # w_hh in DoubleRow-interleaved fp8 layout:
    #   whdr[p, mu*256 + i*128 + g] = w_hh.T[i*128+p, mu*128+g]
    whf = nc.dram_tensor("whf", [128, 8 * 256], FP8, kind="ExternalInput")
    whb = nc.dram_tensor("whb", [128, 8 * 256], FP8, kind="ExternalInput")
    fc1w = nc.dram_tensor("fc1w", [2 * H, 32], FP8, kind="ExternalInput")
    fc1b = nc.dram_tensor("fc1b", [32, 1], F32, kind="ExternalInput")
    fc2w = nc.dram_tensor("fc2w", [32, K], BF16, kind="ExternalInput")
    iden = nc.dram_tensor("iden", [128, 128], F32, kind="ExternalInput")
    # CRF constants (host-prepared):
    #   transrep[b, i*4+j] = trans[i,j] + fc2b[j]        (log domain)
    #   startrep[b, i]     = start_trans[i] + fc2b[i]    (log domain)
    #   eendrep[b, i*4+j]  = exp(end_trans[j])           (exp domain)
    #   eyerep[b, i*4+j]   = 1 if i==j else 0            (exp-domain identity)
    transrep = nc.dram_tensor("transrep", [8, 16], F32, kind="ExternalInput")
    startrep = nc.dram_tensor("startrep", [8, 4], F32, kind="ExternalInput")
    eendrep = nc.dram_tensor("eendrep", [8, 16], F32, kind="ExternalInput")
    eyerep = nc.dram_tensor("eyerep", [8, 16], F32, kind="ExternalInput")
    # tagmaskT[(t%16)*8+b, (t//16)*4+j] = (tags[b,t]==j) * (t==0 ? 1 : m[t,b])
    tagmaskT = nc.dram_tensor("tagmaskT", [128, NCH * 4], F32,
                              kind="ExternalInput")
    out = nc.dram_tensor("out", [1, 1], F32, kind="ExternalOutput")

    n_ttile = TOK // 128          # token tiles of 128
    n_n512 = TOK // 512           # 512-wide token chunks
    ek = [(0, 128), (128, 128), (256, 48)]   # E-chunks (rows of EP)

    with TileContext(nc) as tc:
        with tc.tile_pool(name="const", bufs=1) as constp, \
             tc.tile_pool(name="persist", bufs=1) as pp, \
             tc.tile_pool(name="ps0", bufs=2, space="PSUM") as ps0p:
            # constants in SBUF
            id_sb = constp.tile([128, 128], F32, tag="iden")
            nc.sync.dma_start(id_sb[:], iden[:])
            wx_sb = {}
            for d, src in (("f", wxf), ("b", wxb)):
                for ki, (r0, rn) in enumerate(ek):
                    w = constp.tile([rn, G4H], BF16, tag=f"wx{d}{ki}")
                    nc.sync.dma_start(w[:], src[r0:r0 + rn, :])
                    wx_sb[(d, ki)] = w
            wh_sb = {}
            for d, src in (("f", whf), ("b", whb)):
                w = constp.tile([128, 8 * 256], FP8, tag=f"wh{d}")
                nc.sync.dma_start(w[:], src[:])
                wh_sb[d] = w
            fc1w_sb = []
            for ki in range(4):
                w = constp.tile([128, 32], FP8, tag=f"fc1w{ki}")
                nc.sync.dma_start(w[:], fc1w[ki * 128:(ki + 1) * 128, :])
                fc1w_sb.append(w)
            fc2w_sb = constp.tile([32, K], BF16, tag="fc2w")
            nc.sync.dma_start(fc2w_sb[:], fc2w[:])
            fc1b_sb = constp.tile([32, 1], F32, tag="fc1b")
            nc.sync.dma_start(fc1b_sb[:], fc1b[:])
            transrep_sb = constp.tile([8, 16], F32, tag="transrep")
            nc.sync.dma_start(transrep_sb[:], transrep[:])
            startrep_sb = constp.tile([8, 4], F32, tag="startrep")
            nc.sync.dma_start(startrep_sb[:], startrep[:])
            eendrep_sb = constp.tile([8, 16], F32, tag="eendrep")
            nc.sync.dma_start(eendrep_sb[:], eendrep[:])
            eyerep_sb = constp.tile([8, 16], F32, tag="eyerep")
            nc.sync.dma_start(eyerep_sb[:], eyerep[:])
            tagmaskT_sb = constp.tile([128, NCH * 4], F32, tag="tagmaskT")
            nc.sync.dma_start(tagmaskT_sb[:], tagmaskT[:])
            ones_sb = constp.tile([128, 1], F32, tag="ones")
            nc.vector.memset(ones_sb[:], 1.0)
            id8_sb = constp.tile([128, 128], FP8, tag="id8")
            nc.vector.tensor_copy(out=id8_sb[:], in_=id_sb[:])

            # persistent activations
            # h layout: [128, 2 hid-chunks * TOK], col = k*TOK + t*bc + b
            hT = {d: pp.tile([128, 2 * TOK], FP8, tag=f"h{d}", name=f"h{d}")
                  for d in "fb"}
            # emisTall[(t%16)*8+b, (t//16)*4+j] = emis_raw[t, b, j]
            emisTall = pp.tile([128, NCH * 4], F32, tag="emisTall",
                               name="emisTall")
            # emT2[b, t*4+j] = emis_raw[t, b, j]  (b-major; vector-engine APs
            # must start at partition 0/32/64/96)
            emT2 = pp.tile([8, t_steps * 4], F32, tag="emT2", name="emT2")
            z = pp.tile([32, TOK], BF16, tag="z")

            with tc.tile_pool(name="xg", bufs=1) as xgp, \
                 tc.tile_pool(name="xt", bufs=1) as xtp, \
                 tc.tile_pool(name="xrp", bufs=2) as xrp, \
                 tc.tile_pool(name="rec", bufs=1) as recp, \
                 tc.tile_pool(name="st", bufs=3) as stp, \
                 tc.tile_pool(name="ps2", bufs=2, space="PSUM") as ps2p:
                # xg layout: [128, 8 gate-chunks * TOK], col = mu*TOK + t*bc+b
                xg = {d: xgp.tile([128, 8 * TOK], FP8, tag=f"xg{d}",
                                  name=f"xg{d}") for d in "fb"}
                # ---------- phase 0: gather + transpose -> xT ----------
                xT = [xtp.tile([rn, TOK], BF16, tag=f"xT{ki}", name=f"xT{ki}")
                      for ki, (r0, rn) in enumerate(ek)]
                idx_all = xtp.tile([128, n_ttile], I32, tag="idx_all")
                nc.gpsimd.dma_start(
                    idx_all[:],
                    toks[:].rearrange("(i p) one -> p (i one)", p=128))
                GC = min(8, n_ttile)        # token tiles per gather chunk
                if "0" not in parts:
                    GC = 0
                for c0 in range(0, n_ttile, GC if GC else n_ttile + 1):
                    xr = xrp.tile([128, GC * EP], F32, tag="xr", name="xr")
                    for j in range(GC):
                        i = c0 + j
                        nc.gpsimd.indirect_dma_start(
                            out=xr[:, j * EP:(j + 1) * EP], out_offset=None,
                            in_=emb_aug[:],
                            in_offset=bass.IndirectOffsetOnAxis(
                                ap=idx_all[:, i:i + 1], axis=0),
                        )
                    for j in range(GC):
                        i = c0 + j
                        for ki, (r0, rn) in enumerate(ek):
                            pt = ps0p.tile([128, 128], F32, tag="tp")
                            nc.tensor.transpose(
                                out=pt[:rn, :],
                                in_=xr[:, j * EP + r0:j * EP + r0 + rn],
                                identity=id_sb[:])
                            nc.vector.tensor_copy(
                                out=xT[ki][:, i * 128:(i + 1) * 128],
                                in_=pt[:rn, :])

                c_st = {d: recp.tile([128, 2 * bc], F32, tag=f"c{d}",
                                     name=f"c{d}") for d in "fb"}
                if "1" in parts:
                    for d in "fb":
                        # ----- phase 1: xg = wx^T @ xT (bias via ones-row) --
                        for mu in range(8):
                            for n in range(n_n512):
                                ps = ps0p.tile([128, 512], F32, tag="mm")
                                for ki, (r0, rn) in enumerate(ek):
                                    nc.tensor.matmul(
                                        ps[:],
                                        lhsT=wx_sb[(d, ki)][:, mu * 128:(mu + 1) * 128],
                                        rhs=xT[ki][:, n * 512:(n + 1) * 512],
                                        start=(ki == 0), stop=(ki == 2))
                                nc.scalar.copy(
                                    out=xg[d][:, mu * TOK + n * 512:
                                              mu * TOK + (n + 1) * 512],
                                    in_=ps[:])

                # -------- phase 2: recurrence, fwd+bwd interleaved ----------
                # gate chunk order is i,f,o,g (host permutes weights):
                # sigmoid on [0:6bc], tanh on [6bc:8bc]
                t_rec = t_steps if "2" in parts else 0
                for step in range(t_rec):
                    for d in "fb":
                        t = step if d == "f" else t_steps - 1 - step
                        first = step == 0
                        xga = xg[d][:].rearrange(
                            "p (m t) -> p m t", m=8)[:, :, t * bc:(t + 1) * bc]
                        if first:
                            gp = stp.tile([128, 8 * bc], F32, tag=f"gp{d}",
                                          name=f"gp{d}")
                            nc.vector.tensor_copy(out=gp[:].rearrange(
                                "p (m c) -> p m c", m=8), in_=xga)
                        else:
                            tprev = t - 1 if d == "f" else t + 1
                            ps = ps2p.tile([128, 8 * bc], F32, tag=f"ps{d}",
                                           name=f"ps{d}")
                            hrhs = hT[d][:].rearrange(
                                "p (k t) -> p k t",
                                k=2)[:, :, tprev * bc:(tprev + 1) * bc]
                            for mu in range(8):
                                nc.tensor.matmul(
                                    ps[:, mu * bc:(mu + 1) * bc],
                                    lhsT=wh_sb[d][:, mu * 256:(mu + 1) * 256]
                                    .rearrange("p (two g) -> p two g", two=2),
                                    rhs=hrhs, start=(mu == 0), stop=False,
                                    perf_mode=mybir.MatmulPerfMode.DoubleRow)
                            # fold the xg add into PSUM: += I @ xga
                            nc.tensor.matmul(
                                ps[:].rearrange("p (m c) -> p m c", m=8),
                                lhsT=id8_sb[:], rhs=xga,
                                start=False, stop=True)
                            gp = ps
                        sa = stp.tile([128, 8 * bc], F32, tag=f"sa{d}",
                                      name=f"sa{d}")
                        nc.scalar.activation(sa[:, 0:6 * bc], gp[:, 0:6 * bc],
                                             AF.Sigmoid)
                        nc.scalar.activation(sa[:, 6 * bc:8 * bc],
                                             gp[:, 6 * bc:8 * bc], AF.Tanh)
                        t1 = stp.tile([128, 2 * bc], F32, tag=f"t1{d}",
                                      name=f"t1{d}")
                        t2 = stp.tile([128, 2 * bc], F32, tag=f"t2{d}",
                                      name=f"t2{d}")
                        if first:
                            nc.vector.tensor_tensor(
                                out=c_st[d][:], in0=sa[:, 0:2 * bc],
                                in1=sa[:, 6 * bc:8 * bc], op=ALU.mult)
                        else:
                            nc.vector.tensor_tensor(
                                out=t1[:], in0=sa[:, 2 * bc:4 * bc],
                                in1=c_st[d][:], op=ALU.mult)
                            nc.vector.tensor_tensor(
                                out=t2[:], in0=sa[:, 0:2 * bc],
                                in1=sa[:, 6 * bc:8 * bc], op=ALU.mult)
                            nc.vector.tensor_tensor(
                                out=c_st[d][:], in0=t1[:], in1=t2[:],
                                op=ALU.add)
                        tcl = stp.tile([128, 2 * bc], F32, tag=f"tc{d}",
                                       name=f"tc{d}")
                        nc.scalar.activation(tcl[:], c_st[d][:], AF.Tanh)
                        hdst = hT[d][:].rearrange(
                            "p (k t) -> p k t", k=2)[:, :, t * bc:(t + 1) * bc]
                        nc.vector.tensor_tensor(
                            out=hdst,
                            in0=sa[:, 4 * bc:6 * bc].rearrange(
                                "p (k c) -> p k c", k=2),
                            in1=tcl[:].rearrange("p (k c) -> p k c", k=2),
                            op=ALU.mult)

            # ---------- phase 3: fc1 + relu -> z; emisTall ----------
            n_n512_f = n_n512 if "f" in parts else 0
            for n in range(n_n512_f):
                ps = ps0p.tile([32, 512], F32, tag="mm")
                for ki in range(4):
                    dd = "f" if ki < 2 else "b"
                    kk = ki % 2
                    nc.tensor.matmul(
                        ps[:], lhsT=fc1w_sb[ki],
                        rhs=hT[dd][:, kk * TOK + n * 512:kk * TOK + (n + 1) * 512],
                        start=(ki == 0), stop=(ki == 3))
                nc.scalar.activation(z[:, n * 512:(n + 1) * 512], ps[:],
                                     AF.Relu, bias=fc1b_sb[:, :1])
            # transposed emissions: matmul(lhsT=z chunk [32,128],
            # rhs=fc2w [32,4]) -> [128 tok, 4] (no fc2b: folded into CRF)
            for c in range(NCH if "f" in parts else 0):
                ps = ps0p.tile([128, 128], F32, tag="tp")
                nc.tensor.matmul(ps[:, 0:4],
                                 lhsT=z[:, c * 128:(c + 1) * 128],
                                 rhs=fc2w_sb[:], start=True, stop=True)
                nc.scalar.copy(out=emisTall[:, c * 4:(c + 1) * 4],
                               in_=ps[:, 0:4])

            # ---------- phase 4: CRF on device (tree composition) ----------
            if "c" in parts:
                with tc.tile_pool(name="tree", bufs=1) as treep, \
                     tc.tile_pool(name="cst", bufs=3) as cstp, \
                     tc.tile_pool(name="prod", bufs=2) as prodp:
                    # gold emission score: sum(emisTall * tagmaskT)
                    gprod = treep.tile([128, NCH * 4], F32, tag="gprod")
                    nc.vector.tensor_tensor(out=gprod[:], in0=emisTall[:],
                                            in1=tagmaskT_sb[:], op=ALU.mult)
                    growsum = treep.tile([128, 1], F32, tag="growsum")
                    nc.vector.tensor_reduce(out=growsum[:], in_=gprod[:],
                                            axis=AX.X, op=ALU.add)
                    ngrowsum = treep.tile([128, 1], F32, tag="ngrowsum")
                    nc.vector.tensor_scalar_mul(ngrowsum[:], growsum[:], -1.0)

                    # reshuffle emissions to b-major: 16 SBUF->SBUF DMAs
                    for tcc in range(16):
                        nc.sync.dma_start(
                            emT2[:].rearrange("b (c tcj) -> b c tcj",
                                              tcj=64)[:, :, tcc * 4:
                                                      tcc * 4 + 4],
                            emisTall[tcc * 8:tcc * 8 + 8, :].rearrange(
                                "p (c j) -> p c j", j=4))

                    # leaves: slot s in 0..NM-2 holds M_{t=s+1} in exp domain
                    # with offset noff[s] = -max_j emis[s+1, j]; slot NM-1 = I
                    eMa = treep.tile([8, NM * 16], F32, tag="eMa",
                                     name="eMa")
                    eMb = treep.tile([8, (NM // 2) * 16], F32, tag="eMb",
                                     name="eMb")
                    noA = treep.tile([8, NM], F32, tag="noA", name="noA")
                    noB = treep.tile([8, NM // 2], F32, tag="noB",
                                     name="noB")
                    emn = treep.tile([8, (NM - 1) * 4], F32, tag="emn")

                    nc.vector.tensor_reduce(
                        out=noA[:, 0:NM - 1],
                        in_=emT2[:, 4:4 * NM].rearrange("p (t j) -> p t j",
                                                        j=4),
                        axis=AX.X, op=ALU.max, negate=True)
                    nc.vector.memset(noA[:, NM - 1:NM], 0.0)
                    nc.vector.tensor_tensor(
                        out=emn[:].rearrange("p (t j) -> p t j", j=4),
                        in0=emT2[:, 4:4 * NM].rearrange("p (t j) -> p t j",
                                                        j=4),
                        in1=noA[:, 0:NM - 1].unsqueeze(2).broadcast_to(
                            [8, NM - 1, 4]),
                        op=ALU.add)
                    nc.vector.tensor_tensor(
                        out=eMa[:, 0:(NM - 1) * 16].rearrange(
                            "p (t i j) -> p t i j", i=4, j=4),
                        in0=emn[:].rearrange(
                            "p (t j) -> p t j", j=4).unsqueeze(2).broadcast_to(
                            [8, NM - 1, 4, 4]),
                        in1=transrep_sb[:].rearrange(
                            "p (i j) -> p i j", j=4).unsqueeze(1).broadcast_to(
                            [8, NM - 1, 4, 4]),
                        op=ALU.add)
                    nc.scalar.activation(eMa[:, 0:(NM - 1) * 16],
                                         eMa[:, 0:(NM - 1) * 16], AF.Exp)
                    nc.vector.tensor_copy(out=eMa[:, (NM - 1) * 16:NM * 16],
                                          in_=eyerep_sb[:])

                    # tree levels
                    bufs = [(eMa, noA), (eMb, noB)]
                    nm, cur = NM, 0
                    while nm > 1:
                        nm2 = nm // 2
                        X, noX = bufs[cur]
                        Cb, noC = bufs[1 - cur]
                        nc.vector.tensor_reduce(
                            out=noC[:, 0:nm2],
                            in_=noX[:, 0:nm].rearrange("p (q two) -> p q two",
                                                       two=2),
                            axis=AX.X, op=ALU.add)
                        CHUNK = 64
                        for q0 in range(0, nm2, CHUNK):
                            qn = min(CHUNK, nm2 - q0)
                            prod = prodp.tile([8, CHUNK * 64], F32,
                                              tag="prod", name="prod")
                            # flat layout (q, i, k, m); ISA free-dim APs are
                            # limited to 3D, so loop the i index
                            pv = prod[:, 0:qn * 64].rearrange(
                                "p (q i k m) -> p q i k m", i=4, k=4, m=4)
                            A = X[:, 0:nm * 16].rearrange(
                                "p (q two i m) -> p q two i m",
                                two=2, i=4, m=4)[:, q0:q0 + qn, 0]
                            Bm = X[:, 0:nm * 16].rearrange(
                                "p (q two m k) -> p q two k m",
                                two=2, m=4, k=4)[:, q0:q0 + qn, 1]
                            Cv = Cb[:, q0 * 16:(q0 + qn) * 16].rearrange(
                                "p (q i k) -> p q i k", i=4, k=4)
                            for i in range(4):
                                nc.vector.tensor_tensor(
                                    out=pv[:, :, i],
                                    in0=A[:, :, i].unsqueeze(2).broadcast_to(
                                        [8, qn, 4, 4]),
                                    in1=Bm, op=ALU.mult)
                                nc.vector.tensor_reduce(
                                    out=Cv[:, :, i], in_=pv[:, :, i],
                                    axis=AX.X, op=ALU.add)
                        # per-matrix max-normalization; fold ln(max) into off
                        mx = cstp.tile([8, NM // 2], F32, tag="mx",
                                       name="mx")
                        nc.vector.tensor_reduce(
                            out=mx[:, 0:nm2],
                            in_=Cb[:, 0:nm2 * 16].rearrange(
                                "p (q e) -> p q e", e=16),
                            axis=AX.X, op=ALU.max)
                        rc = cstp.tile([8, NM // 2], F32, tag="rc",
                                       name="rc")
                        nc.vector.reciprocal(out=rc[:, 0:nm2],
                                             in_=mx[:, 0:nm2])
                        nc.vector.tensor_tensor(
                            out=Cb[:, 0:nm2 * 16].rearrange(
                                "p (q e) -> p q e", e=16),
                            in0=Cb[:, 0:nm2 * 16].rearrange(
                                "p (q e) -> p q e", e=16),
                            in1=rc[:, 0:nm2].unsqueeze(2).broadcast_to(
                                [8, nm2, 16]),
                            op=ALU.mult)
                        lm = cstp.tile([8, NM // 2], F32, tag="lm",
                                       name="lm")
                        nc.scalar.activation(lm[:, 0:nm2], mx[:, 0:nm2],
                                             AF.Ln)
                        nc.vector.tensor_tensor(out=noC[:, 0:nm2],
                                                in0=noC[:, 0:nm2],
                                                in1=lm[:, 0:nm2],
                                                op=ALU.subtract)
                        nm, cur = nm2, 1 - cur

                    eT, noT = bufs[cur]
                    # logZ_b = -noT + ln( sum_ij eA0_i * eT[i,j] * eEnd_j )
                    #          - nA0
                    alpha0 = treep.tile([8, 4], F32, tag="alpha0")
                    nc.vector.tensor_tensor(out=alpha0[:], in0=startrep_sb[:],
                                            in1=emT2[:, 0:4], op=ALU.add)
                    nA0 = treep.tile([8, 1], F32, tag="nA0")
                    nc.vector.tensor_reduce(out=nA0[:], in_=alpha0[:],
                                            axis=AX.X, op=ALU.max,
                                            negate=True)
                    eA0 = treep.tile([8, 4], F32, tag="eA0")
                    nc.scalar.activation(eA0[:], alpha0[:], AF.Exp,
                                         bias=nA0[:, :1])
                    w1 = treep.tile([8, 16], F32, tag="w1")
                    nc.vector.tensor_tensor(
                        out=w1[:].rearrange("p (i j) -> p i j", j=4),
                        in0=eA0[:].unsqueeze(2).broadcast_to([8, 4, 4]),
                        in1=eT[:, 0:16].rearrange("p (i j) -> p i j", j=4),
                        op=ALU.mult)
                    w2 = treep.tile([8, 16], F32, tag="w2")
                    nc.vector.tensor_tensor(out=w2[:], in0=w1[:],
                                            in1=eendrep_sb[:], op=ALU.mult)
                    s2 = treep.tile([8, 1], F32, tag="s2")
                    nc.vector.tensor_reduce(out=s2[:], in_=w2[:],
                                            axis=AX.X, op=ALU.add)
                    lg2 = treep.tile([8, 1], F32, tag="lg2")
                    nc.scalar.activation(lg2[:], s2[:], AF.Ln)
                    lz = treep.tile([8, 1], F32, tag="lz")
                    # lz = (lg2 - nA0) - noT
                    nc.vector.scalar_tensor_tensor(
                        out=lz[:], in0=lg2[:], scalar=nA0[:, :1],
                        in1=noT[:, 0:1], op0=ALU.subtract, op1=ALU.subtract)

                    # total = sum_b lz - sum_p growsum  (PE partition-reduce)
                    pst = ps0p.tile([128, 128], F32, tag="tp")
                    nc.tensor.matmul(pst[0:1, 0:1], lhsT=lz[:],
                                     rhs=ones_sb[0:8, :],
                                     start=True, stop=False)
                    nc.tensor.matmul(pst[0:1, 0:1], lhsT=ngrowsum[:],
                                     rhs=ones_sb[:], start=False, stop=True)
                    outv = treep.tile([1, 1], F32, tag="outv")
                    nc.scalar.copy(out=outv[:], in_=pst[0:1, 0:1])
                    nc.sync.dma_start(out[:], outv[:])
            else:
                outv = pp.tile([1, 1], F32, tag="outv")
                nc.vector.memset(outv[:], 0.0)
                nc.sync.dma_start(out[:], outv[:])
    nc.compile()
    return nc


FP8NP = mybir.dt.np(FP8)


def _whdr(w_ih_unused, w_hh, perm):
    """w_hh.T[:, perm] -> DoubleRow-interleaved fp8 [128, 8*256]:
    whdr[p, mu*256 + i*128 + g] = whT[i*128+p, mu*128+g]"""
    whT = np.asarray(w_hh, np.float32).T[:, perm]        # [256, 1024]
    w4 = whT.reshape(2, 128, 8, 128)                     # (i, p, mu, g)
    return np.ascontiguousarray(
        w4.transpose(1, 2, 0, 3).reshape(128, 2048)).astype(FP8NP)


def _prep_shared(emb, w_ih_f, w_hh_f, b_ih_f, b_hh_f, w_ih_b, w_hh_b,
                 b_ih_b, b_hh_b, fc1_w, fc1_b, fc2_w, fc2_b, start_trans,
                 trans, end_trans):
    f32 = np.float32
    emb_aug = np.zeros((V, EP), f32)
    emb_aug[:, :E] = np.asarray(emb, f32)
    emb_aug[0, :E] = 0.0
    emb_aug[:, E] = 1.0

    perm = np.r_[0:512, 768:1024, 512:768]  # i,f,g,o -> i,f,o,g

    def wx(w_ih, b_ih, b_hh):
        m = np.zeros((EP, G4H), f32)
        m[:E, :] = np.asarray(w_ih, f32).T
        m[E, :] = np.asarray(b_ih, f32) + np.asarray(b_hh, f32)
        return m[:, perm].astype(bfloat16).copy()

    fc2b = np.asarray(fc2_b, f32)
    transp = np.asarray(trans, f32) + fc2b[None, :]      # trans'[i,j]
    startp = np.asarray(start_trans, f32) + fc2b         # start'[i]
    transrep = np.tile(transp.reshape(1, 16), (8, 1)).astype(f32)
    eend = np.exp(np.asarray(end_trans, f32))
    eendrep = np.tile(np.tile(eend, 4)[None, :], (8, 1)).astype(f32)
    eyerep = np.tile(np.eye(4, dtype=f32).reshape(1, 16), (8, 1))

    return dict(
        emb_aug=emb_aug,
        wxf=wx(w_ih_f, b_ih_f, b_hh_f),
        wxb=wx(w_ih_b, b_ih_b, b_hh_b),
        whf=_whdr(w_ih_f, w_hh_f, perm),
        whb=_whdr(w_ih_b, w_hh_b, perm),
        fc1w=np.asarray(fc1_w, np.float32).T.astype(FP8NP).copy(),
        fc1b=np.asarray(fc1_b, np.float32).reshape(32, 1).copy(),
        fc2w=np.asarray(fc2_w, np.float32).T.astype(bfloat16).copy(),
        iden=np.eye(128, dtype=np.float32),
        transrep=transrep,
        startrep=np.tile(startp[None, :], (8, 1)).copy(),
        eendrep=eendrep,
        eyerep=eyerep,
    )


def _host_consts(tags, mask, start_trans, trans, end_trans, fc2_b):
    """Per-core tagmaskT inputs + scalar host constant.

    host_const_sum = sum_b [ start[tg0] + fc2b[tg0]
                             + sum_{t>=1} (trans[tg_{t-1},tg_t]
                                           + fc2b[tg_t]) * m_t
                             + end[tg at seq_end] ]
    """
    f32 = np.float32
    tags = np.asarray(tags, np.int64)
    mask = np.asarray(mask)
    m = mask.astype(f32).T                      # [T, B]
    tg = tags.T                                 # [T, B]
    trans = np.asarray(trans, f32)
    start = np.asarray(start_trans, f32)
    end = np.asarray(end_trans, f32)
    fc2b = np.asarray(fc2_b, f32)

    bidx = np.arange(B)
    gold = start[tg[0]] + fc2b[tg[0]]
    gold = gold + ((trans[tg[:-1], tg[1:]] + fc2b[tg[1:]]) * m[1:]).sum(0)
    seq_ends = mask.astype(np.int64).sum(1) - 1
    gold = gold + end[tg[seq_ends, bidx]]
    host_const_sum = float(gold.sum(dtype=np.float64))

    t_arr = np.arange(T)[:, None]               # [T,1]
    b_arr = np.arange(BC)[None, :]              # [1,BC]
    p_idx = (t_arr % 16) * 8 + b_arr            # [T,BC]
    tagmasks = []
    for c in range(NCORES):
        tgc = tg[:, c * BC:(c + 1) * BC]        # [T,BC]
        mc = m[:, c * BC:(c + 1) * BC]
        w = np.where(t_arr == 0, 1.0, mc).astype(f32)
        tm = np.zeros((128, (T // 16) * 4), f32)
        tm[p_idx, (t_arr // 16) * 4 + tgc] = w
        tagmasks.append(tm)
    return tagmasks, host_const_sum


_CACHE = {}


def _make_runner():
    import jax
    from jax.sharding import Mesh, PartitionSpec, NamedSharding
    try:
        from jax.experimental.shard_map import shard_map
    except ImportError:
        from jax import shard_map
    from concourse import bass2jax
    from concourse.bass2jax import _bass_exec_p, partition_id_tensor

    nc = build_bass()
    bass2jax.install_neuronx_cc_hook()
    partition_name = (nc.partition_id_tensor.name
                      if nc.partition_id_tensor else None)
    in_names, out_names, out_avals, zero_outs = [], [], [], []
    for alloc in nc.m.functions[0].allocations:
        if not isinstance(alloc, mybir.MemoryLocationSet):
            continue
        name = alloc.memorylocations[0].name
        if alloc.kind == "ExternalInput":
            if name != partition_name:
                in_names.append(name)
        elif alloc.kind == "ExternalOutput":
            shape = tuple(alloc.tensor_shape)
            dtype = mybir.dt.np(alloc.dtype)
            out_names.append(name)
            out_avals.append(jax.core.ShapedArray(shape, dtype))
            zero_outs.append(np.zeros(shape, dtype))
    n_params = len(in_names)
    in_names_all = in_names + out_names
    if partition_name is not None:
        in_names_all.append(partition_name)

    def _body(*args):
        operands = list(args)
        if partition_name is not None:
            operands.append(partition_id_tensor())
        outs = _bass_exec_p.bind(
            *operands, out_avals=tuple(out_avals),
            in_names=tuple(in_names_all), out_names=tuple(out_names),
            lowering_input_output_aliases=(),
            sim_require_finite=True, sim_require_nnan=True, nc=nc)
        return tuple(outs)

    devices = jax.devices()[:NCORES]
    mesh = Mesh(np.asarray(devices), ("core",))
    # jit1: the bass kernel only (neuronx_cc_hook needs a module that is
    # exactly the bass_exec custom call). jit2: all-reduce of the per-core
    # scalars, compiled by the stock pipeline, so one replicated value can
    # be fetched from a single device.
    sharded = jax.jit(
        shard_map(_body, mesh=mesh,
                  in_specs=(PartitionSpec("core"),) * (n_params + len(out_names)),
                  out_specs=(PartitionSpec("core"),) * len(out_names),
                  check_rep=False),
        keep_unused=True)
    reduce2 = jax.jit(
        shard_map(lambda v: jax.lax.psum(v, "core"), mesh=mesh,
                  in_specs=(PartitionSpec("core"),),
                  out_specs=PartitionSpec(),
                  check_rep=False))
    sh = NamedSharding(mesh, PartitionSpec("core"))
    return dict(jax=jax, sharded=sharded, reduce2=reduce2, sh=sh,
                in_names=in_names, out_names=out_names, zero_outs=zero_outs)


def _run_device(in_maps):
    if "rt" not in _CACHE:
        _CACHE["rt"] = _make_runner()
    rt = _CACHE["rt"]
    jax = rt["jax"]
    concat_in = [np.concatenate([np.asarray(m[n]) for m in in_maps], 0)
                 for n in rt["in_names"]]
    rt["dev_in"] = [jax.device_put(a, rt["sh"]) for a in concat_in]
    rt["dev_zo"] = [jax.device_put(np.concatenate([z] * NCORES, 0), rt["sh"])
                    for z in rt["zero_outs"]]
    return _exec(rt)


def _exec(rt):
    outs = rt["sharded"](*rt["dev_in"], *rt["dev_zo"])
    total = rt["reduce2"](outs[0])
    return float(np.asarray(total)[0, 0])


def _exec_batch(rt, n):
    """Dispatch n independent executions, reduce all their per-core scalars
    on device, fetch once. Returns the n loss totals (list of floats)."""
    jax = rt["jax"]
    key = ("reduceN", n)
    if key not in _CACHE:
        import jax.numpy as jnp
        from jax.sharding import Mesh, PartitionSpec
        try:
            from jax.experimental.shard_map import shard_map
        except ImportError:
            from jax import shard_map
        mesh = Mesh(np.asarray(jax.devices()[:NCORES]), ("core",))

        def f(*vs):
            return jax.lax.psum(jnp.concatenate(vs, 1), "core")

        _CACHE[key] = jax.jit(shard_map(
            f, mesh=mesh, in_specs=(PartitionSpec("core"),) * n,
            out_specs=PartitionSpec(), check_rep=False))
    res = [rt["sharded"](*rt["dev_in"], *rt["dev_zo"])[0] for _ in range(n)]
    vals = np.asarray(_CACHE[key](*res))[0]
    return [float(v) for v in vals]


def _finish(dev_total):
    return np.float32((dev_total - _CACHE["host_const_sum"]) / B)


def kernel_rerun():
    return _finish(_exec(_CACHE["rt"]))


def kernel_rerun_batch(n=32):
    return [_finish(v) for v in _exec_batch(_CACHE["rt"], n)]


def _host_fallback(emb, w_ih_f, w_hh_f, b_ih_f, b_hh_f, w_ih_b, w_hh_b,
                   b_ih_b, b_hh_b, fc1_w, fc1_b, fc2_w, fc2_b, start_trans,
                   trans, end_trans, tokens, tags, mask):
    """Pure numpy reference implementation (general mask support)."""
    f32 = np.float32
    emb0 = np.asarray(emb, f32).copy()
    emb0[0] = 0.0
    x = emb0[np.asarray(tokens)].transpose(1, 0, 2)     # [T,B,E]

    def lstm(w_ih, w_hh, b_ih, b_hh, reverse):
        w_ih = np.asarray(w_ih, f32)
        w_hh = np.asarray(w_hh, f32)
        xg = x @ w_ih.T + np.asarray(b_ih, f32) + np.asarray(b_hh, f32)
        hs = np.zeros((T, B, H), f32)
        h = np.zeros((B, H), f32)
        c = np.zeros((B, H), f32)
        sig = lambda v: 1.0 / (1.0 + np.exp(-v))
        order = range(T - 1, -1, -1) if reverse else range(T)
        for t in order:
            g = xg[t] + h @ w_hh.T
            i, fga, gg, o = np.split(g, 4, axis=-1)
            c = sig(fga) * c + sig(i) * np.tanh(gg)
            h = sig(o) * np.tanh(c)
            hs[t] = h
        return hs

    hf = lstm(w_ih_f, w_hh_f, b_ih_f, b_hh_f, False)
    hb = lstm(w_ih_b, w_hh_b, b_ih_b, b_hh_b, True)
    hcat = np.concatenate([hf, hb], -1)
    z = np.maximum(hcat @ np.asarray(fc1_w, f32).T + np.asarray(fc1_b, f32), 0)
    emis = z @ np.asarray(fc2_w, f32).T + np.asarray(fc2_b, f32)

    trans = np.asarray(trans, np.float64)
    start = np.asarray(start_trans, np.float64)
    end = np.asarray(end_trans, np.float64)
    emis = emis.astype(np.float64)
    tg = np.asarray(tags, np.int64).T
    m = np.asarray(mask, np.float64).T
    bidx = np.arange(B)
    score = start[tg[0]] + emis[0, bidx, tg[0]]
    for t in range(1, T):
        score = score + (trans[tg[t - 1], tg[t]] + emis[t, bidx, tg[t]]) * m[t]
    seq_ends = np.asarray(mask, np.int64).sum(1) - 1
    score = score + end[tg[seq_ends, bidx]]
    alpha = start[None, :] + emis[0]
    for t in range(1, T):
        nxt = alpha[:, :, None] + trans[None] + emis[t][:, None, :]
        mx = nxt.max(axis=1)
        nxt = mx + np.log(np.exp(nxt - mx[:, None, :]).sum(axis=1))
        alpha = np.where(m[t][:, None] > 0, nxt, alpha)
    av = alpha + end[None, :]
    mx = av.max(axis=1)
    logZ = mx + np.log(np.exp(av - mx[:, None]).sum(axis=1))
    return np.float32(-(score - logZ).mean())


def kernel(emb, w_ih_f, w_hh_f, b_ih_f, b_hh_f, w_ih_b, w_hh_b, b_ih_b,
           b_hh_b, fc1_w, fc1_b, fc2_w, fc2_b, start_trans, trans, end_trans,
           tokens, tags, mask):
    if not np.asarray(mask).all():
        # device CRF assumes mask == ones (true for the reference inputs);
        # general masks take the exact host path
        return _host_fallback(emb, w_ih_f, w_hh_f, b_ih_f, b_hh_f, w_ih_b,
                              w_hh_b, b_ih_b, b_hh_b, fc1_w, fc1_b, fc2_w,
                              fc2_b, start_trans, trans, end_trans, tokens,
                              tags, mask)
    shared = _prep_shared(emb, w_ih_f, w_hh_f, b_ih_f, b_hh_f, w_ih_b,
                          w_hh_b, b_ih_b, b_hh_b, fc1_w, fc1_b, fc2_w, fc2_b,
                          start_trans, trans, end_trans)
    tagmasks, host_const_sum = _host_consts(
        tags, mask, start_trans, trans, end_trans, fc2_b)
    _CACHE["host_const_sum"] = host_const_sum
    tokens = np.asarray(tokens)
    in_maps = []
    for c in range(NCORES):
        tk = tokens[c * BC:(c + 1) * BC, :].astype(np.int32)  # [BC, T]
        tk = tk.T.reshape(T * BC, 1).copy()                   # t-major
        in_maps.append({**shared, "toks": tk, "tagmaskT": tagmasks[c]})

    dev_total = _run_device(in_maps)
    return _finish(dev_total)
